# revision 8
# baseline (speedup 1.0000x reference)
"""Bass/Trainium2 kernel for nn_BoxFilter: 9x9 circular box-mean over
(8, 3, 1024, 1024) f32, data-parallel across 8 NeuronCores (1 image/core).

Pipeline per core, per channel, in blocks of 120 output rows:
  - input arrives as bf16 hi/lo pairs (packed host-side during sharding;
    same 4 B/pixel DMA volume as fp32, fp32-accurate after PSUM accumulate)
  - vertical pass: banded ones-matmuls on PE (hi + lo accumulate in PSUM)
  - 1/81 scaling folded into the ACT PSUM->SBUF copy
  - horizontal pass: one DVE tensor_tensor_scan running-box recurrence
    state[t] = state[t-1] + u[t] - u[t-9] over a wrap-padded row buffer
  - loads issue on the Sync HWDGE ring, stores on the Scalar ring, with
    blocks paired into ~1 MB transfers.
"""

import numpy as np
import ml_dtypes

import concourse.bacc as bacc
import concourse.mybir as mybir
import concourse.tile as tile
from concourse.ap import AP
from concourse.bass_utils import run_bass_kernel_spmd

B, C, H, W = 8, 3, 1024, 1024
R = 4            # filter radius
WIN = 2 * R + 1  # 9
AREA = WIN * WIN
MBLK = 120       # output rows per block (input rows = MBLK + 2R = 128)
NBLK = (H + MBLK - 1) // MBLK  # 9 (last block has 64 rows)
UW = WIN + W + 2 * R  # u buffer: [9 zeros | left wrap 4 | row 1024 | right wrap 4]

_CACHE: dict = {}


def _band_weights() -> np.ndarray:
    w = np.zeros((128, MBLK), dtype=ml_dtypes.bfloat16)
    for m in range(MBLK):
        w[m : m + WIN, m] = 1.0
    return w


def _pack_image(x: np.ndarray) -> np.ndarray:
    """[C,H,W] f32 -> [C,H,2,W] bf16 (hi, lo) with hi+lo ~= x."""
    hi = x.astype(ml_dtypes.bfloat16)
    lo = (x - hi.astype(np.float32)).astype(ml_dtypes.bfloat16)
    return np.ascontiguousarray(np.stack([hi, lo], axis=2))


def _build():
    f32 = mybir.dt.float32
    bf16 = mybir.dt.bfloat16
    nc = bacc.Bacc("TRN2", target_bir_lowering=False, debug=False, num_devices=B)
    x_d = nc.dram_tensor("x", [C, H, 2, W], bf16, kind="ExternalInput")
    w_d = nc.dram_tensor("w", [128, MBLK], bf16, kind="ExternalInput")
    o_d = nc.dram_tensor("o", [C, H, W], f32, kind="ExternalOutput")
    # element strides in the packed input (bf16 elements)
    XROW = 2 * W              # one image row = [hi(1024) | lo(1024)]
    XCH = H * XROW            # one channel

    def vertical(v_t, x_t, w_t, m, k, q):
        """v_t[0:m, :] = banded vertical sum of chunk q of x_t (hi+lo)."""
        for n in range(0, W, 512):
            for s in range(2):
                nc.tensor.matmul(
                    v_t[0:m, n : n + 512],
                    w_t[0:k, 0:m],
                    x_t[0:k, q, s * W + n : s * W + n + 512],
                    start=(s == 0),
                    stop=(s == 1),
                )

    def horizontal(o_t, v_t, u_t, m, oq):
        """o_t[0:m, oq, t] = circular 9-wide box mean ending at v col t-9+8;
        output col layout is rotated by 4 (un-rotated by the store DMAs).
        u = [zeros(9) | v/81 (1024) | v[:,0:8]/81]."""
        nc.vector.memset(u_t[0:m, 0:WIN], 0.0)
        nc.scalar.mul(out=u_t[0:m, WIN : WIN + W], in_=v_t[0:m, :], mul=1.0 / AREA)
        nc.vector.tensor_scalar_mul(
            u_t[0:m, WIN + W : UW], v_t[0:m, 0 : 2 * R], 1.0 / AREA
        )
        nc.vector.tensor_tensor_scan(
            out=o_t[0:m, oq, :],
            data0=u_t[0:m, WIN:UW],
            data1=u_t[0:m, 0 : UW - WIN],
            initial=0.0,
            op0=mybir.AluOpType.add,
            op1=mybir.AluOpType.subtract,
        )

    def store(o_t, c, row0, nrow, nblk):
        """Un-rotate: out col w=(t-4)%1024 for scan slot t in [8..1031]."""
        if nblk == 1:
            nc.gpsimd.dma_start(
                o_d.ap()[c, row0 : row0 + nrow, R:W],
                o_t[0:nrow, 0, 2 * R : W + R],
            )
            nc.gpsimd.dma_start(
                o_d.ap()[c, row0 : row0 + nrow, 0:R],
                o_t[0:nrow, 0, W + R : W + 2 * R],
            )
        else:
            nc.gpsimd.dma_start(
                AP(
                    o_d,
                    (c * H + row0) * W + R,
                    [[W, nrow], [MBLK * W, nblk], [1, W - R]],
                ),
                o_t[0:nrow, :, 2 * R : W + R],
            )
            nc.gpsimd.dma_start(
                AP(
                    o_d,
                    (c * H + row0) * W,
                    [[W, nrow], [MBLK * W, nblk], [1, R]],
                ),
                o_t[0:nrow, :, W + R : W + 2 * R],
            )

    def process_pair(c, b0, x_t, q0, w_t, opool, psum, upool):
        o_t = opool.tile([MBLK, 2, W + 2 * R], f32, tag="o2")
        for q in range(2):
            v_t = psum.tile([MBLK, W], f32, tag="v")
            vertical(v_t, x_t, w_t, MBLK, 128, q0 + q)
            u_t = upool.tile([128, UW], f32, tag="u")
            horizontal(o_t, v_t, u_t, MBLK, q)
        store(o_t, c, b0 * MBLK, MBLK, 2)

    with tile.TileContext(nc) as tc:
        with (
            tc.tile_pool(name="wpool", bufs=1) as wpool,
            tc.tile_pool(name="xpool", bufs=2) as xpool,
            tc.tile_pool(name="upool", bufs=8) as upool,
            tc.tile_pool(name="opool", bufs=6) as opool,
            tc.tile_pool(name="psum", bufs=4, space="PSUM") as psum,
        ):
            w_t = wpool.tile([128, MBLK], bf16)
            nc.sync.dma_start(w_t[:], w_d.ap())
            for c in range(C):
                # block 8 first: its small load primes the pipeline
                m, k = H - 8 * MBLK, H - 8 * MBLK + 2 * R
                r0 = 8 * MBLK - R
                x8_t = xpool.tile([128, 1, 2 * W], bf16, tag="x1")
                nc.sync.dma_start(x8_t[0 : H - r0, 0, :], x_d.ap()[c, r0:H, :, :])
                nc.sync.dma_start(
                    x8_t[H - r0 : k, 0, :], x_d.ap()[c, 0 : k - (H - r0), :, :]
                )
                o8_t = opool.tile([MBLK, 1, W + 2 * R], f32, tag="o1")
                v_t = psum.tile([MBLK, W], f32, tag="v")
                vertical(v_t, x8_t, w_t, m, k, 0)
                u_t = upool.tile([128, UW], f32, tag="u")
                horizontal(o8_t, v_t, u_t, m, 0)
                store(o8_t, c, 8 * MBLK, m, 1)
                # pair 0 (blocks 0,1): wrap rows force split loads
                xp0_t = xpool.tile([128, 2, 2 * W], bf16, tag="xp0")
                nc.sync.dma_start(xp0_t[0:R, 0, :], x_d.ap()[c, H - R : H, :, :])
                nc.sync.dma_start(xp0_t[R:128, 0, :], x_d.ap()[c, 0 : 128 - R, :, :])
                nc.sync.dma_start(
                    xp0_t[:, 1, :], x_d.ap()[c, MBLK - R : MBLK - R + 128, :, :]
                )
                process_pair(c, 0, xp0_t, 0, w_t, opool, psum, upool)
                # quad load (blocks 2..5), one 2 MB transfer
                xq_t = xpool.tile([128, 4, 2 * W], bf16, tag="xq")
                nc.sync.dma_start(
                    xq_t[:],
                    AP(
                        x_d,
                        c * XCH + (2 * MBLK - R) * XROW,
                        [[XROW, 128], [MBLK * XROW, 4], [1, XROW]],
                    ),
                )
                process_pair(c, 2, xq_t, 0, w_t, opool, psum, upool)
                process_pair(c, 4, xq_t, 2, w_t, opool, psum, upool)
                # pair (blocks 6,7)
                xp6_t = xpool.tile([128, 2, 2 * W], bf16, tag="xp6")
                nc.sync.dma_start(
                    xp6_t[:],
                    AP(
                        x_d,
                        c * XCH + (6 * MBLK - R) * XROW,
                        [[XROW, 128], [MBLK * XROW, 2], [1, XROW]],
                    ),
                )
                process_pair(c, 6, xp6_t, 0, w_t, opool, psum, upool)
    nc.compile()
    return nc


def _get_nc():
    if "nc" not in _CACHE:
        _CACHE["nc"] = _build()
    return _CACHE["nc"]


def _prepare_in_maps(tensor: np.ndarray) -> list:
    x = np.asarray(tensor, dtype=np.float32)
    assert x.shape == (B, C, H, W), x.shape
    wmat = _band_weights()
    return [{"x": _pack_image(x[i]), "w": wmat} for i in range(B)]


def kernel(tensor: np.ndarray) -> np.ndarray:
    nc = _get_nc()
    in_maps = _prepare_in_maps(tensor)
    res = run_bass_kernel_spmd(nc, in_maps, core_ids=list(range(B)))
    return np.stack([res.results[i]["o"] for i in range(B)], axis=0)
